# revision 1
# baseline (speedup 1.0000x reference)
"""DepthAugmentation Trainium2 kernel.

Reference pipeline (B=64, H=480, W=640, all f32):
  1. noise = bilinear_upsample(noise_lo * sigma, 4x)   (half-pixel centers)
     depth = clip(depth + noise * (depth > 0), 0, 1)
  2. depth *= (dropout_u >= P_DROPOUT)
  3. depth = where(random_u < P_RANDOM, random_vals, depth)
  4. per-sample stick rectangle painted with the (post-step-3) anchor value.

Sharding: pure data parallel, 8 samples per core on 8 NeuronCores.

Per-core device implementation:
  - The 4x bilinear upsample is two dense matmuls on TensorE:
    U1T = N^T @ Av^T  (vertical), then U2 = (U1T)^T @ Ah^T (horizontal).
    Inputs are split hi/lo bf16 so the result is fp32-accurate; the
    upsample weights {.125,.375,.625,.875,1} are exact in bf16.
  - Steps 1-3 are fused elementwise ops spread over DVE/ACT/GpSimd:
      nm   = (d0 > 0) * noise          [scalar_tensor_tensor, reads PSUM]
      t    = d0 + nm                   [tensor_tensor]
      d1   = min(max(t, 0), 1)         [dual-op tensor_scalar]
      q    = (du >= P) * d1            [scalar_tensor_tensor]
      rm   = (ru < P)                  [tensor_scalar on GpSimd, uint8]
      q    = where(rm, rv, q)          [copy_predicated]
  - Sticks: tiny indirect-DMA tail. Anchor pixels are gathered from the
    written output, val = anchor>0 ? anchor : fallback, the <=144 painted
    rows are row-gathered, masked-painted (iota column masks), and
    row-scattered back. All stick geometry arrives as per-core int32
    input tensors, so the single SPMD program serves all cores.
"""

import numpy as np
import ml_dtypes

import concourse.bass as bass
import concourse.tile as tile
from concourse import mybir
from concourse.bass_utils import run_bass_kernel_spmd

F32 = mybir.dt.float32
BF16 = mybir.dt.bfloat16
I32 = mybir.dt.int32
U8 = mybir.dt.uint8
OP = mybir.AluOpType

NOISE_SIGMA = 0.005
P_DROPOUT = 0.003125
P_RANDOM = 0.003125
P_STICK = 0.00025

B, H, W = 64, 480, 640
HL, WL = 120, 160          # noise_lo spatial dims
N_CORES = 8
SPC = B // N_CORES         # samples per core
RPC = SPC * H              # output rows per core block (3840)
IC = 4                     # row chunks (i-chunks) per sample, 120 rows each
ICH = H // IC              # 120
PAD_IDX = 1 << 30          # OOB sentinel for indirect DMA padding
N_PROW = 256               # painted-row slots (2 chunks of 128)


def _upsample_matrix(n_out, n_in):
    """Bilinear upsample matrix, half-pixel centers, edge clamp."""
    A = np.zeros((n_out, n_in), dtype=np.float64)
    scale = n_in / n_out
    for i in range(n_out):
        src = (i + 0.5) * scale - 0.5
        k0 = int(np.floor(src))
        f = src - k0
        A[i, min(max(k0, 0), n_in - 1)] += 1.0 - f
        A[i, min(max(k0 + 1, 0), n_in - 1)] += f
    return A.astype(np.float32)


def _split_multiwaits(nc):
    """This container's walrus build only accepts ONE sync-wait command per
    CTRL instruction; Tile's epilogue drain carries several. Hoist extra
    waits onto single-wait drains inserted just before the offender."""
    for b in nc.m.functions[0].blocks:
        insts = b.instructions
        i = 0
        while i < len(insts):
            inst = insts[i]
            si = inst.sync_info
            if si is not None and si.on_wait is not None and len(si.on_wait) > 1:
                ws = list(si.on_wait)
                while si.on_wait:
                    si.on_wait.pop()
                si.on_wait.append(ws[-1])
                for k, w in enumerate(ws[:-1]):
                    nd = mybir.InstDrain(
                        name=f"{inst.name}-wsplit{k}", ins=[], outs=[]
                    )
                    nd.engine = inst.engine
                    nd.sync_info = mybir.SyncInfo(on_wait=[w], on_update=[])
                    insts.insert(i, nd)
                    nc.inst_map[nd.name] = nd
                    i += 1
            i += 1


def _build_bass(ablate=()):
    nc = bass.Bass(trn_type="TRN2")

    d_dr = nc.dram_tensor("depth", [RPC, W], F32, kind="ExternalInput")
    du_dr = nc.dram_tensor("dropout_u", [RPC, W], F32, kind="ExternalInput")
    ru_dr = nc.dram_tensor("random_u", [RPC, W], F32, kind="ExternalInput")
    rv_dr = nc.dram_tensor("random_vals", [RPC, W], F32, kind="ExternalInput")
    nl_dr = nc.dram_tensor("noise_lo", [SPC * HL, WL], F32, kind="ExternalInput")
    avt_dr = nc.dram_tensor("avt", [HL, H], BF16, kind="ExternalInput")
    aht_dr = nc.dram_tensor("aht", [WL, W], BF16, kind="ExternalInput")
    aidx_dr = nc.dram_tensor("aidx", [SPC, 1], I32, kind="ExternalInput")
    fbv_dr = nc.dram_tensor("fbv", [SPC, 1], F32, kind="ExternalInput")
    prow_dr = nc.dram_tensor("prow", [N_PROW, 1], I32, kind="ExternalInput")
    sprow_dr = nc.dram_tensor("sprow", [N_PROW, 1], I32, kind="ExternalInput")
    pxlo_dr = nc.dram_tensor("pxlo", [N_PROW, 1], F32, kind="ExternalInput")
    pxhi_dr = nc.dram_tensor("pxhi", [N_PROW, 1], F32, kind="ExternalInput")
    out_dr = nc.dram_tensor("out", [RPC, W], F32, kind="ExternalOutput")
    out_flat = out_dr[:].rearrange("a b -> (a b)").unsqueeze(1)

    with tile.TileContext(nc) as tc:
        with (
            tc.tile_pool(name="const", bufs=1) as constp,
            tc.tile_pool(name="din", bufs=3) as din_p,
            tc.tile_pool(name="duin", bufs=2) as duin_p,
            tc.tile_pool(name="ruin", bufs=2) as ruin_p,
            tc.tile_pool(name="rvin", bufs=2) as rvin_p,
            tc.tile_pool(name="nl", bufs=2) as nl_p,
            tc.tile_pool(name="u1", bufs=2) as u1_p,
            tc.tile_pool(name="nm", bufs=3) as nm_p,
            tc.tile_pool(name="q", bufs=3) as q_p,
            tc.tile_pool(name="rm", bufs=2) as rm_p,
            tc.tile_pool(name="stick", bufs=1) as stick_p,
            tc.tile_pool(name="ps1", bufs=2, space="PSUM") as ps1_p,
            tc.tile_pool(name="ps2", bufs=2, space="PSUM") as ps2_p,
            tc.tile_pool(name="ps128", bufs=2, space="PSUM") as ps128_p,
            tc.tile_pool(name="dscr", bufs=1, space="DRAM") as dram_p,
        ):
            # ---- constants
            avt_t = constp.tile([HL, H], BF16)          # (120, 480)
            nc.sync.dma_start(out=avt_t[:], in_=avt_dr[:])
            aht_t0 = constp.tile([80, W], BF16)         # AhT rows 0:80
            aht_t1 = constp.tile([80, W], BF16)         # AhT rows 80:160
            nc.sync.dma_start(out=aht_t0[:], in_=aht_dr[0:80, :])
            nc.sync.dma_start(out=aht_t1[:], in_=aht_dr[80:160, :])
            ident = constp.tile([ICH, ICH], F32)
            from concourse.masks import make_identity
            make_identity(nc, ident[:])
            colidx_i = constp.tile([128, W], I32)
            nc.gpsimd.iota(colidx_i[:], pattern=[[1, W]], base=0, channel_multiplier=0)
            colidx = constp.tile([128, W], F32)
            nc.vector.tensor_copy(colidx[:], colidx_i[:])
            pbias = constp.tile([128, 1], F32)
            nc.vector.memset(pbias[:], float(P_RANDOM))
            one_b = constp.tile([128, 1], F32)
            nc.vector.memset(one_b[:], 1.0)

            out_dmas = []
            for s in range(SPC):
                r0 = s * H
                blk = lambda dr: dr[r0:r0 + H, :].rearrange(
                    "(p r) j -> p r j", p=ICH
                )
                c3 = lambda t: t[:].rearrange("p (r j) -> p r j", r=IC)
                hblk = lambda dr, h: dr[r0:r0 + H, :].rearrange(
                    "(p g r) j -> p g r j", g=2, r=2
                )[:, h]
                h3 = lambda t, h: t[:, 1280 * h:1280 * (h + 1)].rearrange(
                    "p (r j) -> p r j", r=2
                )
                # ---- input DMAs (two 614KB DMAs per tensor per sample)
                d0 = din_p.tile([ICH, IC * W], F32)
                du = duin_p.tile([ICH, IC * W], F32)
                ru = ruin_p.tile([ICH, IC * W], F32)
                rv = rvin_p.tile([ICH, IC * W], F32)
                for _h in range(2):
                    nc.sync.dma_start(out=h3(d0, _h), in_=hblk(d_dr, _h))
                    nc.sync.dma_start(out=h3(du, _h), in_=hblk(du_dr, _h))
                    nc.sync.dma_start(out=h3(ru, _h), in_=hblk(ru_dr, _h))
                    nc.sync.dma_start(out=h3(rv, _h), in_=hblk(rv_dr, _h))
                nlf = nl_p.tile([HL, WL], F32)
                nc.sync.dma_start(out=nlf[:], in_=nl_dr[s * HL:(s + 1) * HL, :])

                if "nocompute" in ablate:
                    dma = nc.scalar.dma_start(out=blk(out_dr), in_=c3(d0))
                    out_dmas.append(dma)
                    continue

                # ---- stage 1: vertical upsample U1T = N^T @ Av^T (hi/lo bf16)
                nhi = nl_p.tile([HL, WL], BF16)
                nc.scalar.copy(nhi[:], nlf[:])
                nlo = nl_p.tile([HL, WL], BF16)
                nc.vector.tensor_tensor(
                    out=nlo[:], in0=nlf[:], in1=nhi[:], op=OP.subtract
                )
                u1hi = []
                u1lo = []
                for c in range(2):
                    ps1 = ps1_p.tile([80, H], F32)
                    nc.tensor.matmul(
                        ps1[:], lhsT=nhi[:, 80 * c:80 * (c + 1)], rhs=avt_t[:],
                        start=True, stop=False,
                    )
                    nc.tensor.matmul(
                        ps1[:], lhsT=nlo[:, 80 * c:80 * (c + 1)], rhs=avt_t[:],
                        start=False, stop=True,
                    )
                    uh = u1_p.tile([80, H], BF16, tag=f"u1hi{c}")
                    nc.scalar.mul(uh[:], ps1[:], NOISE_SIGMA)
                    ul = u1_p.tile([80, H], BF16, tag=f"u1lo{c}")
                    nc.vector.scalar_tensor_tensor(
                        out=ul[:], in0=ps1[:], scalar=NOISE_SIGMA, in1=uh[:],
                        op0=OP.mult, op1=OP.subtract,
                    )
                    u1hi.append(uh)
                    u1lo.append(ul)

                # ---- stage 2 (horizontal upsample) + d0 accumulate.
                # SBUF data tiles are row-grouped: partition p holds rows
                # 4p..4p+3 of the sample (one contiguous 10KB DMA run per
                # partition). Noise for in-group row rho comes from the
                # strided weight slice U1T[:, rho::4]. PSUM per sample is 5
                # banks: four 512-col regions (banks 0-3) + four 128-col
                # regions packed in bank 4 (ordering-chained: start=True
                # clears that bank's has_written bits).
                # PSUM ends up holding t = d0 + upsampled noise; ACT
                # evacuates w = relu(1-t), then c = relu(1-w) = clip01(t).
                # Gates follow: clip(d0+noise*[d0>0])*[du>=P] ==
                #   clip01(d0+noise)*[d0>0]*[du>=P] (exact: 0 when d0==0).
                nm = nm_p.tile([ICH, IC * W], F32)
                u1hi_r = [u[:].rearrange("c (i r) -> c r i", r=IC) for u in u1hi]
                u1lo_r = [u[:].rearrange("c (i r) -> c r i", r=IC) for u in u1lo]
                q = q_p.tile([ICH, IC * W], F32)
                rm = rm_p.tile([ICH, IC * W], U8, tag="rm")

                def emit_region(psl, rho, j0, jw):
                    first = True
                    for uu_r in (u1hi_r, u1lo_r):
                        for c, aht_c in ((0, aht_t0), (1, aht_t1)):
                            nc.tensor.matmul(
                                psl, lhsT=uu_r[c][:, rho, :],
                                rhs=aht_c[:, j0:j0 + jw],
                                start=first, stop=False,
                            )
                            first = False
                    nc.tensor.matmul(
                        psl, lhsT=ident[:],
                        rhs=d0[:, rho * W + j0:rho * W + j0 + jw],
                        start=False, stop=True,
                    )
                    nc.scalar.activation(
                        out=nm[:, rho * W + j0:rho * W + j0 + jw], in_=psl,
                        func=mybir.ActivationFunctionType.Relu,
                        scale=-1.0, bias=one_b[:ICH, :1],
                    )

                for half in range(2):
                    ps = ps2_p.tile([ICH, 1024], F32)   # 2 full banks
                    for lr in range(2):
                        rho = 2 * half + lr
                        emit_region(ps[:, lr * 512:lr * 512 + 512], rho, 0, 512)
                        ps1r = ps128_p.tile([ICH, 128], F32)
                        emit_region(ps1r[:], rho, 512, 128)

                    # ---- per-half elementwise chain on (120, 1280)
                    hsl = slice(1280 * half, 1280 * half + 1280)
                    nc.scalar.activation(
                        out=nm[:, hsl], in_=nm[:, hsl],
                        func=mybir.ActivationFunctionType.Relu,
                        scale=-1.0, bias=one_b[:ICH, :1],
                    )
                    nc.vector.scalar_tensor_tensor(
                        out=nm[:, hsl], in0=d0[:, hsl], scalar=0.0,
                        in1=nm[:, hsl], op0=OP.is_gt, op1=OP.mult,
                    )
                    nc.vector.scalar_tensor_tensor(
                        out=q[:, hsl], in0=du[:, hsl], scalar=P_DROPOUT,
                        in1=nm[:, hsl], op0=OP.is_ge, op1=OP.mult,
                    )
                    if "nocp" not in ablate:
                        nc.vector.tensor_scalar(
                            out=rm[:, hsl], in0=ru[:, hsl], scalar1=P_RANDOM,
                            scalar2=None, op0=OP.is_lt,
                        )
                        nc.vector.copy_predicated(
                            out=q[:, hsl], mask=rm[:, hsl], data=rv[:, hsl]
                        )
                    dma = nc.scalar.dma_start(
                        out=hblk(out_dr, half), in_=h3(q, half)
                    )
                    out_dmas.append(dma)

            # ---- stick tail ----
            if "notail" not in ablate:
                aidx_t = stick_p.tile([SPC, 1], I32)
                nc.sync.dma_start(out=aidx_t[:], in_=aidx_dr[:])
                fbv_t = stick_p.tile([SPC, 1], F32)
                nc.sync.dma_start(out=fbv_t[:], in_=fbv_dr[:])
                nch = N_PROW // 128
                stk3 = lambda dr: dr[:].rearrange("(c p) u -> p c u", c=nch)
                stk3s = lambda t: t[:].rearrange("p (c u) -> p c u", c=nch)
                prow_t = stick_p.tile([128, nch], I32)
                nc.sync.dma_start(out=stk3s(prow_t), in_=stk3(prow_dr))
                sprow_t = stick_p.tile([128, nch], I32)
                nc.sync.dma_start(out=stk3s(sprow_t), in_=stk3(sprow_dr))
                pxlo_t = stick_p.tile([128, nch], F32)
                nc.sync.dma_start(out=stk3s(pxlo_t), in_=stk3(pxlo_dr))
                pxhi_t = stick_p.tile([128, nch], F32)
                nc.sync.dma_start(out=stk3s(pxhi_t), in_=stk3(pxhi_dr))

                # anchor values from the written output
                anch = stick_p.tile([SPC, 1], F32)
                ga = nc.gpsimd.indirect_dma_start(
                    out=anch[:], out_offset=None, in_=out_flat,
                    in_offset=bass.IndirectOffsetOnAxis(ap=aidx_t[:, :1], axis=0),
                )
                for d in out_dmas:
                    tile.add_dep_helper(ga.ins, d.ins)
                m8 = stick_p.tile([SPC, 1], U8)
                nc.vector.tensor_scalar(
                    out=m8[:], in0=anch[:], scalar1=0.0, scalar2=None, op0=OP.is_gt
                )
                val = stick_p.tile([SPC, 1], F32)
                nc.vector.tensor_copy(val[:], fbv_t[:])
                nc.vector.copy_predicated(out=val[:], mask=m8[:], data=anch[:])
                vscr = dram_p.tile([SPC, 1], F32)
                nc.sync.dma_start(out=vscr[:], in_=val[:])

                for ch in range(nch):
                    vrow = stick_p.tile([128, 1], F32, tag=f"vrow{ch}")
                    nc.gpsimd.indirect_dma_start(
                        out=vrow[:], out_offset=None, in_=vscr[:],
                        in_offset=bass.IndirectOffsetOnAxis(ap=sprow_t[:, ch:ch + 1], axis=0),
                    )
                    g = stick_p.tile([128, W], F32, tag=f"g{ch}")
                    nc.vector.memset(g[:], 0.0)
                    gr = nc.gpsimd.indirect_dma_start(
                        out=g[:], out_offset=None, in_=out_dr[:],
                        in_offset=bass.IndirectOffsetOnAxis(ap=prow_t[:, ch:ch + 1], axis=0),
                        bounds_check=RPC - 1, oob_is_err=False,
                    )
                    for d in out_dmas:
                        tile.add_dep_helper(gr.ins, d.ins)
                    cm1 = stick_p.tile([128, W], U8, tag=f"cm1{ch}")
                    nc.vector.tensor_scalar(
                        out=cm1[:], in0=colidx[:], scalar1=pxlo_t[:, ch:ch + 1],
                        scalar2=None, op0=OP.is_ge,
                    )
                    cm = stick_p.tile([128, W], U8, tag=f"cm{ch}")
                    nc.vector.scalar_tensor_tensor(
                        out=cm[:], in0=colidx[:], scalar=pxhi_t[:, ch:ch + 1], in1=cm1[:],
                        op0=OP.is_lt, op1=OP.mult,
                    )
                    nc.vector.copy_predicated(
                        out=g[:], mask=cm[:], data=vrow[:].to_broadcast([128, W])
                    )
                    nc.gpsimd.indirect_dma_start(
                        out=out_dr[:],
                        out_offset=bass.IndirectOffsetOnAxis(ap=prow_t[:, ch:ch + 1], axis=0),
                        in_=g[:], in_offset=None,
                        bounds_check=RPC - 1, oob_is_err=False,
                    )

    _split_multiwaits(nc)
    return nc


def _stick_params(stick_len, stick_width, stick_y, stick_x, horiz_u, stick_u):
    """Vectorized reference stick geometry (ints, host side)."""
    length = stick_len.astype(np.int64) + 1
    width = stick_width.astype(np.int64) + 1
    horiz = horiz_u > 0.5
    span_h = np.where(horiz, width, length)
    span_w = np.where(horiz, length, width)
    y = np.clip(stick_y.astype(np.int64), 0, np.maximum(H - span_h, 1) - 1)
    x = np.clip(stick_x.astype(np.int64), 0, np.maximum(W - span_w, 1) - 1)
    stick_on = stick_u < np.float32(P_STICK * H * W)
    return y, x, span_h, span_w, stick_on


_NC_CACHE = []


def kernel(**inputs):
    depth = np.ascontiguousarray(np.asarray(inputs["depth"], dtype=np.float32))
    noise_lo = np.ascontiguousarray(np.asarray(inputs["noise_lo"], dtype=np.float32))
    dropout_u = np.ascontiguousarray(np.asarray(inputs["dropout_u"], dtype=np.float32))
    random_u = np.ascontiguousarray(np.asarray(inputs["random_u"], dtype=np.float32))
    random_vals = np.ascontiguousarray(
        np.asarray(inputs["random_vals"], dtype=np.float32)
    )
    stick_u = np.asarray(inputs["stick_u"], dtype=np.float32)
    horiz_u = np.asarray(inputs["horiz_u"], dtype=np.float32)
    fallback_vals = np.asarray(inputs["fallback_vals"], dtype=np.float32)
    stick_len = np.asarray(inputs["stick_len"])
    stick_width = np.asarray(inputs["stick_width"])
    stick_y = np.asarray(inputs["stick_y"])
    stick_x = np.asarray(inputs["stick_x"])

    avt = _upsample_matrix(H, HL).T.astype(ml_dtypes.bfloat16)  # (120, 480)
    aht = _upsample_matrix(W, WL).T.astype(ml_dtypes.bfloat16)  # (160, 640)

    y, x, span_h, span_w, stick_on = _stick_params(
        stick_len, stick_width, stick_y, stick_x, horiz_u, stick_u
    )

    in_maps = []
    for k in range(N_CORES):
        s0 = k * SPC
        sl = slice(s0, s0 + SPC)
        prow = np.full((N_PROW, 1), PAD_IDX, np.int32)
        sprow = np.zeros((N_PROW, 1), np.int32)
        pxlo = np.zeros((N_PROW, 1), np.float32)
        pxhi = np.zeros((N_PROW, 1), np.float32)
        aidx = np.zeros((SPC, 1), np.int32)
        n = 0
        for s in range(SPC):
            b = s0 + s
            aidx[s, 0] = (s * H + y[b]) * W + x[b]
            if not stick_on[b]:
                continue
            for r in range(int(span_h[b])):
                prow[n, 0] = s * H + y[b] + r
                sprow[n, 0] = s
                pxlo[n, 0] = x[b]
                pxhi[n, 0] = x[b] + span_w[b]
                n += 1
        in_maps.append({
            "depth": depth[sl].reshape(RPC, W),
            "dropout_u": dropout_u[sl].reshape(RPC, W),
            "random_u": random_u[sl].reshape(RPC, W),
            "random_vals": random_vals[sl].reshape(RPC, W),
            "noise_lo": noise_lo[sl].reshape(SPC * HL, WL),
            "avt": avt,
            "aht": aht,
            "aidx": aidx,
            "fbv": fallback_vals[sl].reshape(SPC, 1),
            "prow": prow,
            "sprow": sprow,
            "pxlo": pxlo,
            "pxhi": pxhi,
        })

    if not _NC_CACHE:
        _NC_CACHE.append(_build_bass())
    nc = _NC_CACHE[0]
    res = run_bass_kernel_spmd(nc, in_maps, core_ids=list(range(N_CORES)))
    out = np.empty((B, 1, H, W), np.float32)
    for k in range(N_CORES):
        out[k * SPC:(k + 1) * SPC, 0] = res.results[k]["out"].reshape(SPC, H, W)
    return out



# revision 2
# speedup vs baseline: 12.6233x; 12.6233x over previous
"""DepthAugmentation Trainium2 kernel.

Reference pipeline (B=64, H=480, W=640, all f32):
  1. noise = bilinear_upsample(noise_lo * sigma, 4x)   (half-pixel centers)
     depth = clip(depth + noise * (depth > 0), 0, 1)
  2. depth *= (dropout_u >= P_DROPOUT)
  3. depth = where(random_u < P_RANDOM, random_vals, depth)
  4. per-sample stick rectangle painted with the (post-step-3) anchor value.

Sharding: pure data parallel, 8 samples per core on 8 NeuronCores.

The kernel is HBM-bound (~245 GB/s/core sustained when all 8 cores
stream), so depth / random_vals / noise_lo are shipped to the device as
bf16 (worst-case output error ~2e-3 abs, well inside the 2e-2 gate).
dropout_u / random_u stay f32: they feed threshold compares whose
decisions must match the reference bit-exactly.

Per-core device implementation:
  - The 4x bilinear upsample is two dense bf16 matmuls on TensorE:
    U1T = N^T @ Av^T  (vertical), then U2 = (U1T)^T @ Ah^T (horizontal).
    The upsample weights {.125,.375,.625,.875,1} are exact in bf16.
  - SBUF data tiles are row-grouped: partition p holds sample rows
    4p..4p+3 (one contiguous 5-10KB DMA run per partition). Noise for
    in-group row rho comes from the strided slice U1T[:, rho::4]. PSUM
    accumulates noise + d0 (identity matmul); ACT evacuates
    w = relu(1-t), then c = relu(1-w) = clip01(t). Gates follow:
      clip(d0+noise*[d0>0])*[du>=P] == clip01(d0+noise)*[d0>0]*[du>=P]
    (exact: result is 0 when d0==0).
  - DVE: gate multiplies + predicated random-value insert.
    GpSimd (otherwise idle): random mask (ru<P) and rv bf16->f32 upcast.
  - Sticks: host sends, per painted output row, the row index, the
    painted column range, the flat anchor-pixel index and the fallback
    value. Column masks are built before the stream ends; the tail is
    gather(anchor)+gather(rows) -> predicated paint -> scatter.
"""

import numpy as np
import ml_dtypes

import concourse.bass as bass
import concourse.tile as tile
from concourse import mybir
from concourse.bass_utils import run_bass_kernel_spmd

F32 = mybir.dt.float32
BF16 = mybir.dt.bfloat16
I32 = mybir.dt.int32
U8 = mybir.dt.uint8
OP = mybir.AluOpType

NOISE_SIGMA = 0.005
P_DROPOUT = 0.003125
P_RANDOM = 0.003125
P_STICK = 0.00025

B, H, W = 64, 480, 640
HL, WL = 120, 160          # noise_lo spatial dims
N_CORES = 8
SPC = B // N_CORES         # samples per core
RPC = SPC * H              # output rows per core block (3840)
IC = 4                     # rows per partition group
ICH = H // IC              # 120 partitions per sample
PAD_IDX = 1 << 30          # OOB sentinel for indirect DMA padding
N_PROW = 256               # painted-row slots (2 chunks of 128)


def _upsample_matrix(n_out, n_in):
    """Bilinear upsample matrix, half-pixel centers, edge clamp."""
    A = np.zeros((n_out, n_in), dtype=np.float64)
    scale = n_in / n_out
    for i in range(n_out):
        src = (i + 0.5) * scale - 0.5
        k0 = int(np.floor(src))
        f = src - k0
        A[i, min(max(k0, 0), n_in - 1)] += 1.0 - f
        A[i, min(max(k0 + 1, 0), n_in - 1)] += f
    return A.astype(np.float32)


def _split_multiwaits(nc):
    """This container's walrus build only accepts ONE sync-wait command per
    CTRL instruction; Tile's epilogue drain carries several. Hoist extra
    waits onto single-wait drains inserted just before the offender."""
    for b in nc.m.functions[0].blocks:
        insts = b.instructions
        i = 0
        while i < len(insts):
            inst = insts[i]
            si = inst.sync_info
            if si is not None and si.on_wait is not None and len(si.on_wait) > 1:
                ws = list(si.on_wait)
                while si.on_wait:
                    si.on_wait.pop()
                si.on_wait.append(ws[-1])
                for k, w in enumerate(ws[:-1]):
                    nd = mybir.InstDrain(
                        name=f"{inst.name}-wsplit{k}", ins=[], outs=[]
                    )
                    nd.engine = inst.engine
                    nd.sync_info = mybir.SyncInfo(on_wait=[w], on_update=[])
                    insts.insert(i, nd)
                    nc.inst_map[nd.name] = nd
                    i += 1
            i += 1


def _build_bass():
    nc = bass.Bass(trn_type="TRN2")

    d_dr = nc.dram_tensor("depth", [RPC, W], BF16, kind="ExternalInput")
    du_dr = nc.dram_tensor("dropout_u", [RPC, W], F32, kind="ExternalInput")
    ru_dr = nc.dram_tensor("random_u", [RPC, W], F32, kind="ExternalInput")
    rv_dr = nc.dram_tensor("random_vals", [RPC, W], BF16, kind="ExternalInput")
    nl_dr = nc.dram_tensor("noise_lo", [SPC * HL, WL], BF16, kind="ExternalInput")
    avt_dr = nc.dram_tensor("avt", [HL, H], BF16, kind="ExternalInput")
    aht_dr = nc.dram_tensor("aht", [WL, W], BF16, kind="ExternalInput")
    prow_dr = nc.dram_tensor("prow", [N_PROW, 1], I32, kind="ExternalInput")
    arow_dr = nc.dram_tensor("arow", [N_PROW, 1], I32, kind="ExternalInput")
    frow_dr = nc.dram_tensor("frow", [N_PROW, 1], F32, kind="ExternalInput")
    pxlo_dr = nc.dram_tensor("pxlo", [N_PROW, 1], F32, kind="ExternalInput")
    pxhi_dr = nc.dram_tensor("pxhi", [N_PROW, 1], F32, kind="ExternalInput")
    out_dr = nc.dram_tensor("out", [RPC, W], F32, kind="ExternalOutput")
    out_flat = out_dr[:].rearrange("a b -> (a b)").unsqueeze(1)

    with tile.TileContext(nc) as tc:
        with (
            tc.tile_pool(name="const", bufs=1) as constp,
            tc.tile_pool(name="din", bufs=2) as din_p,
            tc.tile_pool(name="duin", bufs=2) as duin_p,
            tc.tile_pool(name="ruin", bufs=2) as ruin_p,
            tc.tile_pool(name="rvin", bufs=2) as rvin_p,
            tc.tile_pool(name="nl", bufs=2) as nl_p,
            tc.tile_pool(name="u1", bufs=2) as u1_p,
            tc.tile_pool(name="nm", bufs=2) as nm_p,
            tc.tile_pool(name="q", bufs=2) as q_p,
            tc.tile_pool(name="rm", bufs=2) as rm_p,
            tc.tile_pool(name="rvf", bufs=2) as rvf_p,
            tc.tile_pool(name="stick", bufs=1) as stick_p,
            tc.tile_pool(name="ps1", bufs=2, space="PSUM") as ps1_p,
            tc.tile_pool(name="ps2", bufs=2, space="PSUM") as ps2_p,
            tc.tile_pool(name="ps128", bufs=2, space="PSUM") as ps128_p,
        ):
            # ---- constants
            avt_t = constp.tile([HL, H], BF16)          # (120, 480)
            nc.sync.dma_start(out=avt_t[:], in_=avt_dr[:])
            aht_t0 = constp.tile([80, W], BF16)         # AhT rows 0:80
            aht_t1 = constp.tile([80, W], BF16)         # AhT rows 80:160
            nc.sync.dma_start(out=aht_t0[:], in_=aht_dr[0:80, :])
            nc.sync.dma_start(out=aht_t1[:], in_=aht_dr[80:160, :])
            ident = constp.tile([ICH, ICH], BF16)
            from concourse.masks import make_identity
            make_identity(nc, ident[:])
            colidx_i = constp.tile([128, W], I32)
            nc.gpsimd.iota(colidx_i[:], pattern=[[1, W]], base=0, channel_multiplier=0)
            colidx = constp.tile([128, W], F32)
            nc.vector.tensor_copy(colidx[:], colidx_i[:])
            one_b = constp.tile([128, 1], F32)
            nc.vector.memset(one_b[:], 1.0)

            # ---- stick constants + precomputed column masks
            nch = N_PROW // 128
            stk3 = lambda dr: dr[:].rearrange("(c p) u -> p c u", c=nch)
            stk3s = lambda t: t[:].rearrange("p (c u) -> p c u", c=nch)
            prow_t = stick_p.tile([128, nch], I32)
            nc.sync.dma_start(out=stk3s(prow_t), in_=stk3(prow_dr))
            arow_t = stick_p.tile([128, nch], I32)
            nc.sync.dma_start(out=stk3s(arow_t), in_=stk3(arow_dr))
            frow_t = stick_p.tile([128, nch], F32)
            nc.sync.dma_start(out=stk3s(frow_t), in_=stk3(frow_dr))
            pxlo_t = stick_p.tile([128, nch], F32)
            nc.sync.dma_start(out=stk3s(pxlo_t), in_=stk3(pxlo_dr))
            pxhi_t = stick_p.tile([128, nch], F32)
            nc.sync.dma_start(out=stk3s(pxhi_t), in_=stk3(pxhi_dr))
            cms = []
            for ch in range(nch):
                cm1 = stick_p.tile([128, W], U8, tag=f"cm1{ch}")
                nc.vector.tensor_scalar(
                    out=cm1[:], in0=colidx[:], scalar1=pxlo_t[:, ch:ch + 1],
                    scalar2=None, op0=OP.is_ge,
                )
                cm = stick_p.tile([128, W], U8, tag=f"cm{ch}")
                nc.vector.scalar_tensor_tensor(
                    out=cm[:], in0=colidx[:], scalar=pxhi_t[:, ch:ch + 1], in1=cm1[:],
                    op0=OP.is_lt, op1=OP.mult,
                )
                cms.append(cm)

            out_dmas = []
            for s in range(SPC):
                r0 = s * H
                blk = lambda dr: dr[r0:r0 + H, :].rearrange(
                    "(p r) j -> p r j", p=ICH
                )
                c3 = lambda t: t[:].rearrange("p (r j) -> p r j", r=IC)
                hblk = lambda dr, h: dr[r0:r0 + H, :].rearrange(
                    "(p g r) j -> p g r j", g=2, r=2
                )[:, h]
                h3 = lambda t, h: t[:, 1280 * h:1280 * (h + 1)].rearrange(
                    "p (r j) -> p r j", r=2
                )
                # ---- input DMAs (one whole-sample DMA per tensor)
                d0 = din_p.tile([ICH, IC * W], BF16)
                du = duin_p.tile([ICH, IC * W], F32)
                ru = ruin_p.tile([ICH, IC * W], F32)
                rv = rvin_p.tile([ICH, IC * W], BF16)
                nc.sync.dma_start(out=c3(d0), in_=blk(d_dr))
                nc.sync.dma_start(out=c3(du), in_=blk(du_dr))
                nc.sync.dma_start(out=c3(ru), in_=blk(ru_dr))
                nc.sync.dma_start(out=c3(rv), in_=blk(rv_dr))
                nlf = nl_p.tile([HL, WL], BF16)
                nc.sync.dma_start(out=nlf[:], in_=nl_dr[s * HL:(s + 1) * HL, :])

                # ---- stage 1: vertical upsample U1T = N^T @ Av^T
                u1 = []
                for c in range(2):
                    ps1 = ps1_p.tile([80, H], F32)
                    nc.tensor.matmul(
                        ps1[:], lhsT=nlf[:, 80 * c:80 * (c + 1)], rhs=avt_t[:],
                        start=True, stop=True,
                    )
                    uh = u1_p.tile([80, H], BF16, tag=f"u1{c}")
                    nc.scalar.mul(uh[:], ps1[:], NOISE_SIGMA)
                    u1.append(uh)

                # ---- stage 2 (horizontal upsample) + d0 accumulate
                nm = nm_p.tile([ICH, IC * W], F32)
                u1_r = [u[:].rearrange("c (i r) -> c r i", r=IC) for u in u1]
                q = q_p.tile([ICH, IC * W], F32)

                def emit_region(psl, rho, j0, jw):
                    for k, (c, aht_c) in enumerate(((0, aht_t0), (1, aht_t1))):
                        nc.tensor.matmul(
                            psl, lhsT=u1_r[c][:, rho, :],
                            rhs=aht_c[:, j0:j0 + jw],
                            start=(k == 0), stop=False,
                        )
                    nc.tensor.matmul(
                        psl, lhsT=ident[:],
                        rhs=d0[:, rho * W + j0:rho * W + j0 + jw],
                        start=False, stop=True,
                    )
                    nc.scalar.activation(
                        out=nm[:, rho * W + j0:rho * W + j0 + jw], in_=psl,
                        func=mybir.ActivationFunctionType.Relu,
                        scale=-1.0, bias=one_b[:ICH, :1],
                    )

                for half in range(2):
                    ps = ps2_p.tile([ICH, 1024], F32)   # 2 full banks
                    for lr in range(2):
                        rho = 2 * half + lr
                        emit_region(ps[:, lr * 512:lr * 512 + 512], rho, 0, 512)
                        ps1r = ps128_p.tile([ICH, 128], F32)
                        emit_region(ps1r[:], rho, 512, 128)

                    # ---- per-half elementwise chain on (120, 1280)
                    hsl = slice(1280 * half, 1280 * half + 1280)
                    nc.scalar.activation(
                        out=nm[:, hsl], in_=nm[:, hsl],
                        func=mybir.ActivationFunctionType.Relu,
                        scale=-1.0, bias=one_b[:ICH, :1],
                    )
                    nc.vector.scalar_tensor_tensor(
                        out=nm[:, hsl], in0=d0[:, hsl], scalar=0.0,
                        in1=nm[:, hsl], op0=OP.is_gt, op1=OP.mult,
                    )
                    nc.vector.scalar_tensor_tensor(
                        out=q[:, hsl], in0=du[:, hsl], scalar=P_DROPOUT,
                        in1=nm[:, hsl], op0=OP.is_ge, op1=OP.mult,
                    )
                    rm = rm_p.tile([ICH, 1280], U8, tag=f"rm{half}")
                    nc.gpsimd.tensor_scalar(
                        out=rm[:], in0=ru[:, hsl], scalar1=P_RANDOM,
                        scalar2=None, op0=OP.is_lt,
                    )
                    rvf = rvf_p.tile([ICH, 1280], F32, tag=f"rvf{half}")
                    nc.gpsimd.tensor_copy(rvf[:], rv[:, hsl])
                    nc.vector.copy_predicated(
                        out=q[:, hsl], mask=rm[:], data=rvf[:]
                    )
                    dma = nc.scalar.dma_start(
                        out=hblk(out_dr, half), in_=h3(q, half)
                    )
                    out_dmas.append(dma)

            # ---- stick tail: anchor gather -> paint -> scatter
            for ch in range(nch):
                anch = stick_p.tile([128, 1], F32, tag=f"anch{ch}")
                nc.vector.memset(anch[:], 0.0)
                ga = nc.gpsimd.indirect_dma_start(
                    out=anch[:], out_offset=None, in_=out_flat,
                    in_offset=bass.IndirectOffsetOnAxis(ap=arow_t[:, ch:ch + 1], axis=0),
                    bounds_check=RPC * W - 1, oob_is_err=False,
                )
                for d in out_dmas:
                    tile.add_dep_helper(ga.ins, d.ins)
                m8 = stick_p.tile([128, 1], U8, tag=f"m8{ch}")
                nc.vector.tensor_scalar(
                    out=m8[:], in0=anch[:], scalar1=0.0, scalar2=None, op0=OP.is_gt
                )
                val = stick_p.tile([128, 1], F32, tag=f"val{ch}")
                nc.vector.tensor_copy(val[:], frow_t[:, ch:ch + 1])
                nc.vector.copy_predicated(out=val[:], mask=m8[:], data=anch[:])

                g = stick_p.tile([128, W], F32, tag=f"g{ch}")
                nc.vector.memset(g[:], 0.0)
                gr = nc.gpsimd.indirect_dma_start(
                    out=g[:], out_offset=None, in_=out_dr[:],
                    in_offset=bass.IndirectOffsetOnAxis(ap=prow_t[:, ch:ch + 1], axis=0),
                    bounds_check=RPC - 1, oob_is_err=False,
                )
                for d in out_dmas:
                    tile.add_dep_helper(gr.ins, d.ins)
                nc.vector.copy_predicated(
                    out=g[:], mask=cms[ch][:], data=val[:].to_broadcast([128, W])
                )
                nc.gpsimd.indirect_dma_start(
                    out=out_dr[:],
                    out_offset=bass.IndirectOffsetOnAxis(ap=prow_t[:, ch:ch + 1], axis=0),
                    in_=g[:], in_offset=None,
                    bounds_check=RPC - 1, oob_is_err=False,
                )

    _split_multiwaits(nc)
    return nc


def _stick_params(stick_len, stick_width, stick_y, stick_x, horiz_u, stick_u):
    """Vectorized reference stick geometry (ints, host side)."""
    length = stick_len.astype(np.int64) + 1
    width = stick_width.astype(np.int64) + 1
    horiz = horiz_u > 0.5
    span_h = np.where(horiz, width, length)
    span_w = np.where(horiz, length, width)
    y = np.clip(stick_y.astype(np.int64), 0, np.maximum(H - span_h, 1) - 1)
    x = np.clip(stick_x.astype(np.int64), 0, np.maximum(W - span_w, 1) - 1)
    stick_on = stick_u < np.float32(P_STICK * H * W)
    return y, x, span_h, span_w, stick_on


_NC_CACHE = []


def kernel(**inputs):
    depth = np.asarray(inputs["depth"], dtype=np.float32)
    noise_lo = np.asarray(inputs["noise_lo"], dtype=np.float32)
    dropout_u = np.ascontiguousarray(np.asarray(inputs["dropout_u"], dtype=np.float32))
    random_u = np.ascontiguousarray(np.asarray(inputs["random_u"], dtype=np.float32))
    random_vals = np.asarray(inputs["random_vals"], dtype=np.float32)
    stick_u = np.asarray(inputs["stick_u"], dtype=np.float32)
    horiz_u = np.asarray(inputs["horiz_u"], dtype=np.float32)
    fallback_vals = np.asarray(inputs["fallback_vals"], dtype=np.float32)
    stick_len = np.asarray(inputs["stick_len"])
    stick_width = np.asarray(inputs["stick_width"])
    stick_y = np.asarray(inputs["stick_y"])
    stick_x = np.asarray(inputs["stick_x"])

    bf16 = ml_dtypes.bfloat16
    depth_b = np.ascontiguousarray(depth.astype(bf16))
    rv_b = np.ascontiguousarray(random_vals.astype(bf16))
    nl_b = np.ascontiguousarray(noise_lo.astype(bf16))

    avt = _upsample_matrix(H, HL).T.astype(bf16)  # (120, 480)
    aht = _upsample_matrix(W, WL).T.astype(bf16)  # (160, 640)

    y, x, span_h, span_w, stick_on = _stick_params(
        stick_len, stick_width, stick_y, stick_x, horiz_u, stick_u
    )

    in_maps = []
    for k in range(N_CORES):
        s0 = k * SPC
        sl = slice(s0, s0 + SPC)
        prow = np.full((N_PROW, 1), PAD_IDX, np.int32)
        arow = np.full((N_PROW, 1), PAD_IDX, np.int32)
        frow = np.zeros((N_PROW, 1), np.float32)
        pxlo = np.zeros((N_PROW, 1), np.float32)
        pxhi = np.zeros((N_PROW, 1), np.float32)
        n = 0
        for s in range(SPC):
            b = s0 + s
            if not stick_on[b]:
                continue
            for r in range(int(span_h[b])):
                prow[n, 0] = s * H + y[b] + r
                arow[n, 0] = (s * H + y[b]) * W + x[b]
                frow[n, 0] = fallback_vals[b]
                pxlo[n, 0] = x[b]
                pxhi[n, 0] = x[b] + span_w[b]
                n += 1
        in_maps.append({
            "depth": depth_b[sl].reshape(RPC, W),
            "dropout_u": dropout_u[sl].reshape(RPC, W),
            "random_u": random_u[sl].reshape(RPC, W),
            "random_vals": rv_b[sl].reshape(RPC, W),
            "noise_lo": nl_b[sl].reshape(SPC * HL, WL),
            "avt": avt,
            "aht": aht,
            "prow": prow,
            "arow": arow,
            "frow": frow,
            "pxlo": pxlo,
            "pxhi": pxhi,
        })

    if not _NC_CACHE:
        _NC_CACHE.append(_build_bass())
    nc = _NC_CACHE[0]
    res = run_bass_kernel_spmd(nc, in_maps, core_ids=list(range(N_CORES)))
    out = np.empty((B, 1, H, W), np.float32)
    for k in range(N_CORES):
        out[k * SPC:(k + 1) * SPC, 0] = res.results[k]["out"].reshape(SPC, H, W)
    return out


# revision 6
# speedup vs baseline: 27.0254x; 2.1409x over previous
"""DepthAugmentation Trainium2 kernel.

Reference pipeline (B=64, H=480, W=640, all f32):
  1. noise = bilinear_upsample(noise_lo * sigma, 4x)   (half-pixel centers)
     depth = clip(depth + noise * (depth > 0), 0, 1)
  2. depth *= (dropout_u >= P_DROPOUT)
  3. depth = where(random_u < P_RANDOM, random_vals, depth)
  4. per-sample stick rectangle painted with the (post-step-3) anchor value.

Sharding: pure data parallel, 8 samples per core on 8 NeuronCores.

The kernel is HBM-bound (~245 GB/s/core sustained when all 8 cores
stream), so depth / random_vals / noise_lo are shipped to the device as
bf16 (worst-case output error ~2e-3 abs, well inside the 2e-2 gate).
dropout_u / random_u stay f32: they feed threshold compares whose
decisions must match the reference bit-exactly.

Per-core device implementation:
  - The 4x bilinear upsample is two dense bf16 matmuls on TensorE:
    U1T = N^T @ Av^T  (vertical), then U2 = (U1T)^T @ Ah^T (horizontal).
    The upsample weights {.125,.375,.625,.875,1} are exact in bf16.
  - SBUF data tiles are row-grouped: partition p holds sample rows
    4p..4p+3 (one contiguous 5-10KB DMA run per partition). Noise for
    in-group row rho comes from the strided slice U1T[:, rho::4]. PSUM
    accumulates noise + d0 (identity matmul); ACT evacuates
    w = relu(1-t), then c = relu(1-w) = clip01(t). The (depth > 0)
    validity gate is free: invalid pixels arrive encoded as -1024, so
    clip01 already yields 0 there. DVE then applies the dropout gate
    and the predicated random-value insert. (GpSimd streaming is ~10x
    slower than DVE and stalls DVE via the shared SBUF port - keep all
    bulk elementwise work off it.)
  - Sticks: host sends, per painted output row, the row index, the
    painted column range, the flat anchor-pixel index and the fallback
    value. Column masks are built before the stream ends; the tail is
    gather(anchor)+gather(rows) -> predicated paint -> scatter.
"""

import numpy as np
import ml_dtypes

import concourse.bass as bass
import concourse.tile as tile
from concourse import mybir
from concourse.bass_utils import run_bass_kernel_spmd

F32 = mybir.dt.float32
BF16 = mybir.dt.bfloat16
I32 = mybir.dt.int32
U8 = mybir.dt.uint8
OP = mybir.AluOpType

NOISE_SIGMA = 0.005
P_DROPOUT = 0.003125
P_RANDOM = 0.003125
P_STICK = 0.00025

B, H, W = 64, 480, 640
HL, WL = 120, 160          # noise_lo spatial dims
N_CORES = 8
SPC = B // N_CORES         # samples per core
RPC = SPC * H              # output rows per core block (3840)
IC = 4                     # rows per partition group
ICH = H // IC              # 120 partitions per sample
PAD_IDX = 1 << 30          # OOB sentinel for indirect DMA padding
N_PROW = 256               # painted-row slots (2 chunks of 128)


def _upsample_matrix(n_out, n_in):
    """Bilinear upsample matrix, half-pixel centers, edge clamp."""
    A = np.zeros((n_out, n_in), dtype=np.float64)
    scale = n_in / n_out
    for i in range(n_out):
        src = (i + 0.5) * scale - 0.5
        k0 = int(np.floor(src))
        f = src - k0
        A[i, min(max(k0, 0), n_in - 1)] += 1.0 - f
        A[i, min(max(k0 + 1, 0), n_in - 1)] += f
    return A.astype(np.float32)


def _split_multiwaits(nc):
    """This container's walrus build only accepts ONE sync-wait command per
    CTRL instruction; Tile's epilogue drain carries several. Hoist extra
    waits onto single-wait drains inserted just before the offender."""
    for b in nc.m.functions[0].blocks:
        insts = b.instructions
        i = 0
        while i < len(insts):
            inst = insts[i]
            si = inst.sync_info
            if si is not None and si.on_wait is not None and len(si.on_wait) > 1:
                ws = list(si.on_wait)
                while si.on_wait:
                    si.on_wait.pop()
                si.on_wait.append(ws[-1])
                for k, w in enumerate(ws[:-1]):
                    nd = mybir.InstDrain(
                        name=f"{inst.name}-wsplit{k}", ins=[], outs=[]
                    )
                    nd.engine = inst.engine
                    nd.sync_info = mybir.SyncInfo(on_wait=[w], on_update=[])
                    insts.insert(i, nd)
                    nc.inst_map[nd.name] = nd
                    i += 1
            i += 1


def _build_bass():
    nc = bass.Bass(trn_type="TRN2")

    d_dr = nc.dram_tensor("depth", [RPC, W], BF16, kind="ExternalInput")
    du_dr = nc.dram_tensor("dropout_u", [RPC, W], F32, kind="ExternalInput")
    ru_dr = nc.dram_tensor("random_u", [RPC, W], F32, kind="ExternalInput")
    rv_dr = nc.dram_tensor("random_vals", [RPC, W], BF16, kind="ExternalInput")
    nl_dr = nc.dram_tensor("noise_lo", [SPC * HL, WL], BF16, kind="ExternalInput")
    avt_dr = nc.dram_tensor("avt", [HL, H], BF16, kind="ExternalInput")
    aht_dr = nc.dram_tensor("aht", [WL, W], BF16, kind="ExternalInput")
    prow_dr = nc.dram_tensor("prow", [N_PROW, 1], I32, kind="ExternalInput")
    arow_dr = nc.dram_tensor("arow", [N_PROW, 1], I32, kind="ExternalInput")
    frow_dr = nc.dram_tensor("frow", [N_PROW, 1], F32, kind="ExternalInput")
    pxlo_dr = nc.dram_tensor("pxlo", [N_PROW, 1], F32, kind="ExternalInput")
    pxhi_dr = nc.dram_tensor("pxhi", [N_PROW, 1], F32, kind="ExternalInput")
    out_dr = nc.dram_tensor("out", [RPC, W], F32, kind="ExternalOutput")
    out_flat = out_dr[:].rearrange("a b -> (a b)").unsqueeze(1)

    with tile.TileContext(nc) as tc:
        with (
            tc.tile_pool(name="const", bufs=1) as constp,
            tc.tile_pool(name="din", bufs=2) as din_p,
            tc.tile_pool(name="duin", bufs=2) as duin_p,
            tc.tile_pool(name="ruin", bufs=2) as ruin_p,
            tc.tile_pool(name="rvin", bufs=2) as rvin_p,
            tc.tile_pool(name="nl", bufs=2) as nl_p,
            tc.tile_pool(name="u1", bufs=2) as u1_p,
            tc.tile_pool(name="nm", bufs=2) as nm_p,
            tc.tile_pool(name="q", bufs=2) as q_p,
            tc.tile_pool(name="rm", bufs=2) as rm_p,
            tc.tile_pool(name="stick", bufs=1) as stick_p,
            tc.tile_pool(name="ps1", bufs=2, space="PSUM") as ps1_p,
            tc.tile_pool(name="ps2", bufs=2, space="PSUM") as ps2_p,
            tc.tile_pool(name="ps128", bufs=2, space="PSUM") as ps128_p,
        ):
            # ---- constants
            avt_t = constp.tile([HL, H], BF16)          # (120, 480)
            nc.sync.dma_start(out=avt_t[:], in_=avt_dr[:])
            aht_t0 = constp.tile([80, W], BF16)         # AhT rows 0:80
            aht_t1 = constp.tile([80, W], BF16)         # AhT rows 80:160
            nc.sync.dma_start(out=aht_t0[:], in_=aht_dr[0:80, :])
            nc.sync.dma_start(out=aht_t1[:], in_=aht_dr[80:160, :])
            ident = constp.tile([ICH, ICH], BF16)
            from concourse.masks import make_identity
            make_identity(nc, ident[:])
            colidx_i = constp.tile([128, W], I32)
            nc.gpsimd.iota(colidx_i[:], pattern=[[1, W]], base=0, channel_multiplier=0)
            colidx = constp.tile([128, W], F32)
            nc.vector.tensor_copy(colidx[:], colidx_i[:])
            one_b = constp.tile([128, 1], F32)
            nc.vector.memset(one_b[:], 1.0)

            # ---- stick constants + precomputed column masks
            nch = N_PROW // 128
            stk3 = lambda dr: dr[:].rearrange("(c p) u -> p c u", c=nch)
            stk3s = lambda t: t[:].rearrange("p (c u) -> p c u", c=nch)
            prow_t = stick_p.tile([128, nch], I32)
            nc.sync.dma_start(out=stk3s(prow_t), in_=stk3(prow_dr))
            arow_t = stick_p.tile([128, nch], I32)
            nc.sync.dma_start(out=stk3s(arow_t), in_=stk3(arow_dr))
            frow_t = stick_p.tile([128, nch], F32)
            nc.sync.dma_start(out=stk3s(frow_t), in_=stk3(frow_dr))
            pxlo_t = stick_p.tile([128, nch], F32)
            nc.sync.dma_start(out=stk3s(pxlo_t), in_=stk3(pxlo_dr))
            pxhi_t = stick_p.tile([128, nch], F32)
            nc.sync.dma_start(out=stk3s(pxhi_t), in_=stk3(pxhi_dr))
            cms = []
            for ch in range(nch):
                cm1 = stick_p.tile([128, W], U8, tag=f"cm1{ch}")
                nc.vector.tensor_scalar(
                    out=cm1[:], in0=colidx[:], scalar1=pxlo_t[:, ch:ch + 1],
                    scalar2=None, op0=OP.is_ge,
                )
                cm = stick_p.tile([128, W], U8, tag=f"cm{ch}")
                nc.vector.scalar_tensor_tensor(
                    out=cm[:], in0=colidx[:], scalar=pxhi_t[:, ch:ch + 1], in1=cm1[:],
                    op0=OP.is_lt, op1=OP.mult,
                )
                cms.append(cm)

            out_dmas = []
            for s in range(SPC):
                r0 = s * H
                blk = lambda dr: dr[r0:r0 + H, :].rearrange(
                    "(p r) j -> p r j", p=ICH
                )
                c3 = lambda t: t[:].rearrange("p (r j) -> p r j", r=IC)
                hblk = lambda dr, h: dr[r0:r0 + H, :].rearrange(
                    "(p g r) j -> p g r j", g=2, r=2
                )[:, h]
                h3 = lambda t, h: t[:, 1280 * h:1280 * (h + 1)].rearrange(
                    "p (r j) -> p r j", r=2
                )
                # ---- input DMAs (one whole-sample DMA per tensor)
                d0 = din_p.tile([ICH, IC * W], BF16)
                du = duin_p.tile([ICH, IC * W], F32)
                ru = ruin_p.tile([ICH, IC * W], F32)
                rv = rvin_p.tile([ICH, IC * W], BF16)
                nc.sync.dma_start(out=c3(d0), in_=blk(d_dr))
                nc.sync.dma_start(out=c3(du), in_=blk(du_dr))
                nc.sync.dma_start(out=c3(ru), in_=blk(ru_dr))
                nc.sync.dma_start(out=c3(rv), in_=blk(rv_dr))
                nlf = nl_p.tile([HL, WL], BF16)
                nc.sync.dma_start(out=nlf[:], in_=nl_dr[s * HL:(s + 1) * HL, :])

                # ---- stage 1: vertical upsample U1T = N^T @ Av^T
                u1 = []
                for c in range(2):
                    ps1 = ps1_p.tile([80, H], F32)
                    nc.tensor.matmul(
                        ps1[:], lhsT=nlf[:, 80 * c:80 * (c + 1)], rhs=avt_t[:],
                        start=True, stop=True,
                    )
                    uh = u1_p.tile([80, H], BF16, tag=f"u1{c}")
                    nc.scalar.mul(uh[:], ps1[:], NOISE_SIGMA)
                    u1.append(uh)

                # ---- stage 2 (horizontal upsample) + d0 accumulate
                nm = nm_p.tile([ICH, IC * W], F32)
                u1_r = [u[:].rearrange("c (i r) -> c r i", r=IC) for u in u1]
                q = q_p.tile([ICH, IC * W], F32)

                def emit_region(psl, rho, j0, jw):
                    for k, (c, aht_c) in enumerate(((0, aht_t0), (1, aht_t1))):
                        nc.tensor.matmul(
                            psl, lhsT=u1_r[c][:, rho, :],
                            rhs=aht_c[:, j0:j0 + jw],
                            start=(k == 0), stop=False,
                        )
                    nc.tensor.matmul(
                        psl, lhsT=ident[:],
                        rhs=d0[:, rho * W + j0:rho * W + j0 + jw],
                        start=False, stop=True,
                    )
                    nc.scalar.activation(
                        out=nm[:, rho * W + j0:rho * W + j0 + jw], in_=psl,
                        func=mybir.ActivationFunctionType.Relu,
                        scale=-1.0, bias=one_b[:ICH, :1],
                    )

                for half in range(2):
                    ps = ps2_p.tile([ICH, 1024], F32)   # 2 full banks
                    for lr in range(2):
                        rho = 2 * half + lr
                        emit_region(ps[:, lr * 512:lr * 512 + 512], rho, 0, 512)
                        ps1r = ps128_p.tile([ICH, 128], F32)
                        emit_region(ps1r[:], rho, 512, 128)

                    # ---- per-half elementwise chain on (120, 1280)
                    hsl = slice(1280 * half, 1280 * half + 1280)
                    nc.scalar.activation(
                        out=nm[:, hsl], in_=nm[:, hsl],
                        func=mybir.ActivationFunctionType.Relu,
                        scale=-1.0, bias=one_b[:ICH, :1],
                    )
                    rm = rm_p.tile([ICH, 1280], U8, tag=f"rm{half}")
                    nc.vector.tensor_scalar(
                        out=rm[:], in0=ru[:, hsl], scalar1=P_RANDOM,
                        scalar2=None, op0=OP.is_lt,
                    )
                    nc.vector.scalar_tensor_tensor(
                        out=q[:, hsl], in0=du[:, hsl], scalar=P_DROPOUT,
                        in1=nm[:, hsl], op0=OP.is_ge, op1=OP.mult,
                    )
                    nc.vector.copy_predicated(
                        out=q[:, hsl], mask=rm[:], data=rv[:, hsl]
                    )
                    dma = nc.scalar.dma_start(
                        out=hblk(out_dr, half), in_=h3(q, half)
                    )
                    out_dmas.append(dma)

            # ---- stick tail: anchor gather -> paint -> scatter
            for ch in range(nch):
                anch = stick_p.tile([128, 1], F32, tag=f"anch{ch}")
                nc.vector.memset(anch[:], 0.0)
                ga = nc.gpsimd.indirect_dma_start(
                    out=anch[:], out_offset=None, in_=out_flat,
                    in_offset=bass.IndirectOffsetOnAxis(ap=arow_t[:, ch:ch + 1], axis=0),
                    bounds_check=RPC * W - 1, oob_is_err=False,
                )
                for d in out_dmas:
                    tile.add_dep_helper(ga.ins, d.ins)
                m8 = stick_p.tile([128, 1], U8, tag=f"m8{ch}")
                nc.vector.tensor_scalar(
                    out=m8[:], in0=anch[:], scalar1=0.0, scalar2=None, op0=OP.is_gt
                )
                val = stick_p.tile([128, 1], F32, tag=f"val{ch}")
                nc.vector.tensor_copy(val[:], frow_t[:, ch:ch + 1])
                nc.vector.copy_predicated(out=val[:], mask=m8[:], data=anch[:])

                g = stick_p.tile([128, W], F32, tag=f"g{ch}")
                nc.vector.memset(g[:], 0.0)
                gr = nc.gpsimd.indirect_dma_start(
                    out=g[:], out_offset=None, in_=out_dr[:],
                    in_offset=bass.IndirectOffsetOnAxis(ap=prow_t[:, ch:ch + 1], axis=0),
                    bounds_check=RPC - 1, oob_is_err=False,
                )
                for d in out_dmas:
                    tile.add_dep_helper(gr.ins, d.ins)
                nc.vector.copy_predicated(
                    out=g[:], mask=cms[ch][:], data=val[:].to_broadcast([128, W])
                )
                nc.gpsimd.indirect_dma_start(
                    out=out_dr[:],
                    out_offset=bass.IndirectOffsetOnAxis(ap=prow_t[:, ch:ch + 1], axis=0),
                    in_=g[:], in_offset=None,
                    bounds_check=RPC - 1, oob_is_err=False,
                )

    _split_multiwaits(nc)
    return nc


def _stick_params(stick_len, stick_width, stick_y, stick_x, horiz_u, stick_u):
    """Vectorized reference stick geometry (ints, host side)."""
    length = stick_len.astype(np.int64) + 1
    width = stick_width.astype(np.int64) + 1
    horiz = horiz_u > 0.5
    span_h = np.where(horiz, width, length)
    span_w = np.where(horiz, length, width)
    y = np.clip(stick_y.astype(np.int64), 0, np.maximum(H - span_h, 1) - 1)
    x = np.clip(stick_x.astype(np.int64), 0, np.maximum(W - span_w, 1) - 1)
    stick_on = stick_u < np.float32(P_STICK * H * W)
    return y, x, span_h, span_w, stick_on


_NC_CACHE = []


def kernel(**inputs):
    depth = np.asarray(inputs["depth"], dtype=np.float32)
    noise_lo = np.asarray(inputs["noise_lo"], dtype=np.float32)
    dropout_u = np.ascontiguousarray(np.asarray(inputs["dropout_u"], dtype=np.float32))
    random_u = np.ascontiguousarray(np.asarray(inputs["random_u"], dtype=np.float32))
    random_vals = np.asarray(inputs["random_vals"], dtype=np.float32)
    stick_u = np.asarray(inputs["stick_u"], dtype=np.float32)
    horiz_u = np.asarray(inputs["horiz_u"], dtype=np.float32)
    fallback_vals = np.asarray(inputs["fallback_vals"], dtype=np.float32)
    stick_len = np.asarray(inputs["stick_len"])
    stick_width = np.asarray(inputs["stick_width"])
    stick_y = np.asarray(inputs["stick_y"])
    stick_x = np.asarray(inputs["stick_x"])

    bf16 = ml_dtypes.bfloat16
    # Invalid depth (==0) is re-encoded as -1024 (exact in bf16): the
    # device-side clip01(depth + noise) then yields exactly 0 for those
    # pixels, which replaces a per-pixel (depth > 0) gate op on DVE.
    depth_b = np.ascontiguousarray(
        np.where(depth == 0.0, np.float32(-1024.0), depth).astype(bf16)
    )
    rv_b = np.ascontiguousarray(random_vals.astype(bf16))
    nl_b = np.ascontiguousarray(noise_lo.astype(bf16))

    avt = _upsample_matrix(H, HL).T.astype(bf16)  # (120, 480)
    aht = _upsample_matrix(W, WL).T.astype(bf16)  # (160, 640)

    y, x, span_h, span_w, stick_on = _stick_params(
        stick_len, stick_width, stick_y, stick_x, horiz_u, stick_u
    )

    in_maps = []
    for k in range(N_CORES):
        s0 = k * SPC
        sl = slice(s0, s0 + SPC)
        prow = np.full((N_PROW, 1), PAD_IDX, np.int32)
        arow = np.full((N_PROW, 1), PAD_IDX, np.int32)
        frow = np.zeros((N_PROW, 1), np.float32)
        pxlo = np.zeros((N_PROW, 1), np.float32)
        pxhi = np.zeros((N_PROW, 1), np.float32)
        n = 0
        for s in range(SPC):
            b = s0 + s
            if not stick_on[b]:
                continue
            for r in range(int(span_h[b])):
                prow[n, 0] = s * H + y[b] + r
                arow[n, 0] = (s * H + y[b]) * W + x[b]
                frow[n, 0] = fallback_vals[b]
                pxlo[n, 0] = x[b]
                pxhi[n, 0] = x[b] + span_w[b]
                n += 1
        in_maps.append({
            "depth": depth_b[sl].reshape(RPC, W),
            "dropout_u": dropout_u[sl].reshape(RPC, W),
            "random_u": random_u[sl].reshape(RPC, W),
            "random_vals": rv_b[sl].reshape(RPC, W),
            "noise_lo": nl_b[sl].reshape(SPC * HL, WL),
            "avt": avt,
            "aht": aht,
            "prow": prow,
            "arow": arow,
            "frow": frow,
            "pxlo": pxlo,
            "pxhi": pxhi,
        })

    if not _NC_CACHE:
        _NC_CACHE.append(_build_bass())
    nc = _NC_CACHE[0]
    res = run_bass_kernel_spmd(nc, in_maps, core_ids=list(range(N_CORES)))
    out = np.empty((B, 1, H, W), np.float32)
    for k in range(N_CORES):
        out[k * SPC:(k + 1) * SPC, 0] = res.results[k]["out"].reshape(SPC, H, W)
    return out


# revision 7
# speedup vs baseline: 30.2966x; 1.1210x over previous
"""DepthAugmentation Trainium2 kernel.

Reference pipeline (B=64, H=480, W=640, all f32):
  1. noise = bilinear_upsample(noise_lo * sigma, 4x)   (half-pixel centers)
     depth = clip(depth + noise * (depth > 0), 0, 1)
  2. depth *= (dropout_u >= P_DROPOUT)
  3. depth = where(random_u < P_RANDOM, random_vals, depth)
  4. per-sample stick rectangle painted with the (post-step-3) anchor value.

Sharding: pure data parallel, 8 samples per core on 8 NeuronCores.

The kernel is HBM-bound (~245 GB/s/core sustained when all 8 cores
stream), so depth / random_vals / noise_lo are shipped to the device as
bf16 (worst-case output error ~2e-3 abs, well inside the 2e-2 gate).
dropout_u / random_u stay f32: they feed threshold compares whose
decisions must match the reference bit-exactly.

Per-core device implementation:
  - The 4x bilinear upsample is two dense bf16 matmuls on TensorE:
    U1T = N^T @ Av^T  (vertical), then U2 = (U1T)^T @ Ah^T (horizontal).
    The upsample weights {.125,.375,.625,.875,1} are exact in bf16.
  - SBUF data tiles are row-grouped: partition p holds sample rows
    4p..4p+3 (one contiguous 5-10KB DMA run per partition). Noise for
    in-group row rho comes from the strided slice U1T[:, rho::4]. PSUM
    accumulates noise + d0 (identity matmul); ACT evacuates
    w = relu(1-t), then c = relu(1-w) = clip01(t). The (depth > 0)
    validity gate is free: invalid pixels arrive encoded as -1024, so
    clip01 already yields 0 there. DVE then applies the dropout gate
    and the predicated random-value insert. (GpSimd streaming is ~10x
    slower than DVE and stalls DVE via the shared SBUF port - keep all
    bulk elementwise work off it.)
  - Sticks: host sends, per painted output row, the row index, the
    painted column range, the flat anchor-pixel index and the fallback
    value. Column masks are built before the stream ends; the tail is
    gather(anchor)+gather(rows) -> predicated paint -> scatter.
"""

import numpy as np
import ml_dtypes

import concourse.bass as bass
import concourse.tile as tile
from concourse import mybir
from concourse.bass_utils import run_bass_kernel_spmd

F32 = mybir.dt.float32
BF16 = mybir.dt.bfloat16
I32 = mybir.dt.int32
U8 = mybir.dt.uint8
OP = mybir.AluOpType

NOISE_SIGMA = 0.005
P_DROPOUT = 0.003125
P_RANDOM = 0.003125
P_STICK = 0.00025

B, H, W = 64, 480, 640
HL, WL = 120, 160          # noise_lo spatial dims
N_CORES = 8
SPC = B // N_CORES         # samples per core
RPC = SPC * H              # output rows per core block (3840)
IC = 4                     # rows per partition group
ICH = H // IC              # 120 partitions per sample
PAD_IDX = 1 << 30          # OOB sentinel for indirect DMA padding
N_PROW = 256               # painted-row slots (2 chunks of 128)


def _upsample_matrix(n_out, n_in):
    """Bilinear upsample matrix, half-pixel centers, edge clamp."""
    A = np.zeros((n_out, n_in), dtype=np.float64)
    scale = n_in / n_out
    for i in range(n_out):
        src = (i + 0.5) * scale - 0.5
        k0 = int(np.floor(src))
        f = src - k0
        A[i, min(max(k0, 0), n_in - 1)] += 1.0 - f
        A[i, min(max(k0 + 1, 0), n_in - 1)] += f
    return A.astype(np.float32)


def _split_multiwaits(nc):
    """This container's walrus build only accepts ONE sync-wait command per
    CTRL instruction; Tile's epilogue drain carries several. Hoist extra
    waits onto single-wait drains inserted just before the offender."""
    for b in nc.m.functions[0].blocks:
        insts = b.instructions
        i = 0
        while i < len(insts):
            inst = insts[i]
            si = inst.sync_info
            if si is not None and si.on_wait is not None and len(si.on_wait) > 1:
                ws = list(si.on_wait)
                while si.on_wait:
                    si.on_wait.pop()
                si.on_wait.append(ws[-1])
                for k, w in enumerate(ws[:-1]):
                    nd = mybir.InstDrain(
                        name=f"{inst.name}-wsplit{k}", ins=[], outs=[]
                    )
                    nd.engine = inst.engine
                    nd.sync_info = mybir.SyncInfo(on_wait=[w], on_update=[])
                    insts.insert(i, nd)
                    nc.inst_map[nd.name] = nd
                    i += 1
            i += 1


def _build_bass():
    nc = bass.Bass(trn_type="TRN2")

    d_dr = nc.dram_tensor("depth", [RPC, W], BF16, kind="ExternalInput")
    du_dr = nc.dram_tensor("dropout_u", [RPC, W], F32, kind="ExternalInput")
    ru_dr = nc.dram_tensor("random_u", [RPC, W], F32, kind="ExternalInput")
    rv_dr = nc.dram_tensor("random_vals", [RPC, W], BF16, kind="ExternalInput")
    nl_dr = nc.dram_tensor("noise_lo", [SPC * HL, WL], BF16, kind="ExternalInput")
    avt_dr = nc.dram_tensor("avt", [HL, H], BF16, kind="ExternalInput")
    aht_dr = nc.dram_tensor("aht", [WL, W], BF16, kind="ExternalInput")
    prow_dr = nc.dram_tensor("prow", [N_PROW, 1], I32, kind="ExternalInput")
    arow_dr = nc.dram_tensor("arow", [N_PROW, 1], I32, kind="ExternalInput")
    frow_dr = nc.dram_tensor("frow", [N_PROW, 1], F32, kind="ExternalInput")
    pxlo_dr = nc.dram_tensor("pxlo", [N_PROW, 1], F32, kind="ExternalInput")
    pxhi_dr = nc.dram_tensor("pxhi", [N_PROW, 1], F32, kind="ExternalInput")
    out_dr = nc.dram_tensor("out", [RPC, W], F32, kind="ExternalOutput")
    out_flat = out_dr[:].rearrange("a b -> (a b)").unsqueeze(1)

    with tile.TileContext(nc) as tc:
        with (
            tc.tile_pool(name="const", bufs=1) as constp,
            tc.tile_pool(name="din", bufs=3) as din_p,
            tc.tile_pool(name="duin", bufs=3) as duin_p,
            tc.tile_pool(name="ruin", bufs=3) as ruin_p,
            tc.tile_pool(name="rvin", bufs=3) as rvin_p,
            tc.tile_pool(name="nl", bufs=2) as nl_p,
            tc.tile_pool(name="u1", bufs=2) as u1_p,
            tc.tile_pool(name="nm", bufs=2) as nm_p,
            tc.tile_pool(name="q", bufs=2) as q_p,
            tc.tile_pool(name="rm", bufs=2) as rm_p,
            tc.tile_pool(name="stick", bufs=1) as stick_p,
            tc.tile_pool(name="ps1", bufs=2, space="PSUM") as ps1_p,
            tc.tile_pool(name="ps2", bufs=2, space="PSUM") as ps2_p,
            tc.tile_pool(name="ps128", bufs=2, space="PSUM") as ps128_p,
        ):
            # ---- constants
            avt_t = constp.tile([HL, H], BF16)          # (120, 480)
            nc.sync.dma_start(out=avt_t[:], in_=avt_dr[:])
            aht_t0 = constp.tile([80, W], BF16)         # AhT rows 0:80
            aht_t1 = constp.tile([80, W], BF16)         # AhT rows 80:160
            nc.sync.dma_start(out=aht_t0[:], in_=aht_dr[0:80, :])
            nc.sync.dma_start(out=aht_t1[:], in_=aht_dr[80:160, :])
            ident = constp.tile([ICH, ICH], BF16)
            from concourse.masks import make_identity
            make_identity(nc, ident[:])
            colidx_i = constp.tile([128, W], I32)
            nc.gpsimd.iota(colidx_i[:], pattern=[[1, W]], base=0, channel_multiplier=0)
            colidx = constp.tile([128, W], F32)
            nc.vector.tensor_copy(colidx[:], colidx_i[:])
            one_b = constp.tile([128, 1], F32)
            nc.vector.memset(one_b[:], 1.0)

            # ---- stick constants + precomputed column masks
            nch = N_PROW // 128
            stk3 = lambda dr: dr[:].rearrange("(c p) u -> p c u", c=nch)
            stk3s = lambda t: t[:].rearrange("p (c u) -> p c u", c=nch)
            prow_t = stick_p.tile([128, nch], I32)
            nc.sync.dma_start(out=stk3s(prow_t), in_=stk3(prow_dr))
            arow_t = stick_p.tile([128, nch], I32)
            nc.sync.dma_start(out=stk3s(arow_t), in_=stk3(arow_dr))
            frow_t = stick_p.tile([128, nch], F32)
            nc.sync.dma_start(out=stk3s(frow_t), in_=stk3(frow_dr))
            pxlo_t = stick_p.tile([128, nch], F32)
            nc.sync.dma_start(out=stk3s(pxlo_t), in_=stk3(pxlo_dr))
            pxhi_t = stick_p.tile([128, nch], F32)
            nc.sync.dma_start(out=stk3s(pxhi_t), in_=stk3(pxhi_dr))
            cms = []
            for ch in range(nch):
                cm1 = stick_p.tile([128, W], U8, tag=f"cm1{ch}")
                nc.vector.tensor_scalar(
                    out=cm1[:], in0=colidx[:], scalar1=pxlo_t[:, ch:ch + 1],
                    scalar2=None, op0=OP.is_ge,
                )
                cm = stick_p.tile([128, W], U8, tag=f"cm{ch}")
                nc.vector.scalar_tensor_tensor(
                    out=cm[:], in0=colidx[:], scalar=pxhi_t[:, ch:ch + 1], in1=cm1[:],
                    op0=OP.is_lt, op1=OP.mult,
                )
                cms.append(cm)

            out_dmas = []
            for s in range(SPC):
                r0 = s * H
                blk = lambda dr: dr[r0:r0 + H, :].rearrange(
                    "(p r) j -> p r j", p=ICH
                )
                c3 = lambda t: t[:].rearrange("p (r j) -> p r j", r=IC)
                hblk = lambda dr, h: dr[r0:r0 + H, :].rearrange(
                    "(p g r) j -> p g r j", g=2, r=2
                )[:, h]
                h3 = lambda t, h: t[:, 1280 * h:1280 * (h + 1)].rearrange(
                    "p (r j) -> p r j", r=2
                )
                # ---- input DMAs (one whole-sample DMA per tensor)
                d0 = din_p.tile([ICH, IC * W], BF16)
                du = duin_p.tile([ICH, IC * W], F32)
                ru = ruin_p.tile([ICH, IC * W], F32)
                rv = rvin_p.tile([ICH, IC * W], BF16)
                nc.sync.dma_start(out=c3(d0), in_=blk(d_dr))
                nc.sync.dma_start(out=c3(du), in_=blk(du_dr))
                nc.sync.dma_start(out=c3(ru), in_=blk(ru_dr))
                nc.sync.dma_start(out=c3(rv), in_=blk(rv_dr))
                nlf = nl_p.tile([HL, WL], BF16)
                nc.sync.dma_start(out=nlf[:], in_=nl_dr[s * HL:(s + 1) * HL, :])

                # ---- stage 1: vertical upsample U1T = N^T @ Av^T
                u1 = []
                for c in range(2):
                    ps1 = ps1_p.tile([80, H], F32)
                    nc.tensor.matmul(
                        ps1[:], lhsT=nlf[:, 80 * c:80 * (c + 1)], rhs=avt_t[:],
                        start=True, stop=True,
                    )
                    uh = u1_p.tile([80, H], BF16, tag=f"u1{c}")
                    nc.scalar.mul(uh[:], ps1[:], NOISE_SIGMA)
                    u1.append(uh)

                # ---- stage 2 (horizontal upsample) + d0 accumulate
                nm = nm_p.tile([ICH, IC * W], F32)
                u1_r = [u[:].rearrange("c (i r) -> c r i", r=IC) for u in u1]
                q = q_p.tile([ICH, IC * W], F32)

                def emit_region(psl, rho, j0, jw):
                    for k, (c, aht_c) in enumerate(((0, aht_t0), (1, aht_t1))):
                        nc.tensor.matmul(
                            psl, lhsT=u1_r[c][:, rho, :],
                            rhs=aht_c[:, j0:j0 + jw],
                            start=(k == 0), stop=False,
                        )
                    nc.tensor.matmul(
                        psl, lhsT=ident[:],
                        rhs=d0[:, rho * W + j0:rho * W + j0 + jw],
                        start=False, stop=True,
                    )
                    nc.scalar.activation(
                        out=nm[:, rho * W + j0:rho * W + j0 + jw], in_=psl,
                        func=mybir.ActivationFunctionType.Relu,
                        scale=-1.0, bias=one_b[:ICH, :1],
                    )

                for half in range(2):
                    ps = ps2_p.tile([ICH, 1024], F32)   # 2 full banks
                    for lr in range(2):
                        rho = 2 * half + lr
                        emit_region(ps[:, lr * 512:lr * 512 + 512], rho, 0, 512)
                        ps1r = ps128_p.tile([ICH, 128], F32)
                        emit_region(ps1r[:], rho, 512, 128)

                    # ---- per-half elementwise chain on (120, 1280)
                    hsl = slice(1280 * half, 1280 * half + 1280)
                    nc.scalar.activation(
                        out=nm[:, hsl], in_=nm[:, hsl],
                        func=mybir.ActivationFunctionType.Relu,
                        scale=-1.0, bias=one_b[:ICH, :1],
                    )
                    rm = rm_p.tile([ICH, 1280], U8, tag=f"rm{half}")
                    nc.vector.tensor_scalar(
                        out=rm[:], in0=ru[:, hsl], scalar1=P_RANDOM,
                        scalar2=None, op0=OP.is_lt,
                    )
                    nc.vector.scalar_tensor_tensor(
                        out=q[:, hsl], in0=du[:, hsl], scalar=P_DROPOUT,
                        in1=nm[:, hsl], op0=OP.is_ge, op1=OP.mult,
                    )
                    nc.vector.copy_predicated(
                        out=q[:, hsl], mask=rm[:], data=rv[:, hsl]
                    )
                    dma = nc.scalar.dma_start(
                        out=hblk(out_dr, half), in_=h3(q, half)
                    )
                    out_dmas.append(dma)

            # ---- stick tail: anchor gather -> paint -> scatter
            # Slots are pre-assigned per sample (32 per sample), so chunk ch
            # covers samples 4ch..4ch+3 and only waits on their out DMAs:
            # chunk 0's whole chain runs while samples 4-7 still stream.
            for ch in range(nch):
                deps = out_dmas[8 * ch:8 * ch + 8]
                anch = stick_p.tile([128, 1], F32, tag=f"anch{ch}")
                nc.vector.memset(anch[:], 0.0)
                ga = nc.gpsimd.indirect_dma_start(
                    out=anch[:], out_offset=None, in_=out_flat,
                    in_offset=bass.IndirectOffsetOnAxis(ap=arow_t[:, ch:ch + 1], axis=0),
                    bounds_check=RPC * W - 1, oob_is_err=False,
                )
                for d in deps:
                    tile.add_dep_helper(ga.ins, d.ins)
                m8 = stick_p.tile([128, 1], U8, tag=f"m8{ch}")
                nc.vector.tensor_scalar(
                    out=m8[:], in0=anch[:], scalar1=0.0, scalar2=None, op0=OP.is_gt
                )
                val = stick_p.tile([128, 1], F32, tag=f"val{ch}")
                nc.vector.tensor_copy(val[:], frow_t[:, ch:ch + 1])
                nc.vector.copy_predicated(out=val[:], mask=m8[:], data=anch[:])

                g = stick_p.tile([128, W], F32, tag=f"g{ch}")
                nc.vector.memset(g[:], 0.0)
                gr = nc.gpsimd.indirect_dma_start(
                    out=g[:], out_offset=None, in_=out_dr[:],
                    in_offset=bass.IndirectOffsetOnAxis(ap=prow_t[:, ch:ch + 1], axis=0),
                    bounds_check=RPC - 1, oob_is_err=False,
                )
                for d in deps:
                    tile.add_dep_helper(gr.ins, d.ins)
                nc.vector.copy_predicated(
                    out=g[:], mask=cms[ch][:], data=val[:].to_broadcast([128, W])
                )
                nc.gpsimd.indirect_dma_start(
                    out=out_dr[:],
                    out_offset=bass.IndirectOffsetOnAxis(ap=prow_t[:, ch:ch + 1], axis=0),
                    in_=g[:], in_offset=None,
                    bounds_check=RPC - 1, oob_is_err=False,
                )

    _split_multiwaits(nc)
    return nc


def _stick_params(stick_len, stick_width, stick_y, stick_x, horiz_u, stick_u):
    """Vectorized reference stick geometry (ints, host side)."""
    length = stick_len.astype(np.int64) + 1
    width = stick_width.astype(np.int64) + 1
    horiz = horiz_u > 0.5
    span_h = np.where(horiz, width, length)
    span_w = np.where(horiz, length, width)
    y = np.clip(stick_y.astype(np.int64), 0, np.maximum(H - span_h, 1) - 1)
    x = np.clip(stick_x.astype(np.int64), 0, np.maximum(W - span_w, 1) - 1)
    stick_on = stick_u < np.float32(P_STICK * H * W)
    return y, x, span_h, span_w, stick_on


_NC_CACHE = []


def kernel(**inputs):
    depth = np.asarray(inputs["depth"], dtype=np.float32)
    noise_lo = np.asarray(inputs["noise_lo"], dtype=np.float32)
    dropout_u = np.ascontiguousarray(np.asarray(inputs["dropout_u"], dtype=np.float32))
    random_u = np.ascontiguousarray(np.asarray(inputs["random_u"], dtype=np.float32))
    random_vals = np.asarray(inputs["random_vals"], dtype=np.float32)
    stick_u = np.asarray(inputs["stick_u"], dtype=np.float32)
    horiz_u = np.asarray(inputs["horiz_u"], dtype=np.float32)
    fallback_vals = np.asarray(inputs["fallback_vals"], dtype=np.float32)
    stick_len = np.asarray(inputs["stick_len"])
    stick_width = np.asarray(inputs["stick_width"])
    stick_y = np.asarray(inputs["stick_y"])
    stick_x = np.asarray(inputs["stick_x"])

    bf16 = ml_dtypes.bfloat16
    # Invalid depth (==0) is re-encoded as -1024 (exact in bf16): the
    # device-side clip01(depth + noise) then yields exactly 0 for those
    # pixels, which replaces a per-pixel (depth > 0) gate op on DVE.
    depth_b = np.ascontiguousarray(
        np.where(depth == 0.0, np.float32(-1024.0), depth).astype(bf16)
    )
    rv_b = np.ascontiguousarray(random_vals.astype(bf16))
    nl_b = np.ascontiguousarray(noise_lo.astype(bf16))

    avt = _upsample_matrix(H, HL).T.astype(bf16)  # (120, 480)
    aht = _upsample_matrix(W, WL).T.astype(bf16)  # (160, 640)

    y, x, span_h, span_w, stick_on = _stick_params(
        stick_len, stick_width, stick_y, stick_x, horiz_u, stick_u
    )

    in_maps = []
    for k in range(N_CORES):
        s0 = k * SPC
        sl = slice(s0, s0 + SPC)
        prow = np.full((N_PROW, 1), PAD_IDX, np.int32)
        arow = np.full((N_PROW, 1), PAD_IDX, np.int32)
        frow = np.zeros((N_PROW, 1), np.float32)
        pxlo = np.zeros((N_PROW, 1), np.float32)
        pxhi = np.zeros((N_PROW, 1), np.float32)
        slots_per_sample = N_PROW // SPC
        for s in range(SPC):
            b = s0 + s
            if not stick_on[b]:
                continue
            for r in range(int(span_h[b])):
                n = s * slots_per_sample + r
                prow[n, 0] = s * H + y[b] + r
                arow[n, 0] = (s * H + y[b]) * W + x[b]
                frow[n, 0] = fallback_vals[b]
                pxlo[n, 0] = x[b]
                pxhi[n, 0] = x[b] + span_w[b]
        in_maps.append({
            "depth": depth_b[sl].reshape(RPC, W),
            "dropout_u": dropout_u[sl].reshape(RPC, W),
            "random_u": random_u[sl].reshape(RPC, W),
            "random_vals": rv_b[sl].reshape(RPC, W),
            "noise_lo": nl_b[sl].reshape(SPC * HL, WL),
            "avt": avt,
            "aht": aht,
            "prow": prow,
            "arow": arow,
            "frow": frow,
            "pxlo": pxlo,
            "pxhi": pxhi,
        })

    if not _NC_CACHE:
        _NC_CACHE.append(_build_bass())
    nc = _NC_CACHE[0]
    res = run_bass_kernel_spmd(nc, in_maps, core_ids=list(range(N_CORES)))
    out = np.empty((B, 1, H, W), np.float32)
    for k in range(N_CORES):
        out[k * SPC:(k + 1) * SPC, 0] = res.results[k]["out"].reshape(SPC, H, W)
    return out
